# revision 26
# baseline (speedup 1.0000x reference)
"""BinConv2dEval Trainium2 kernel (fp8 DoubleRow version).

y = conv2d(x, W, stride 1, pad 1) + bias ; out = (round(y) * sign >= 0) ? 1 : 0

All values are integers (x in {0,1}, W in {-1,0,1}, bias integer), so round()
is a no-op and everything is exact in fp8e4 matmuls with fp32 PSUM
accumulation. Folding: with s = sign[c] in {+-1},
    (conv + bias) * s >= 0   <=>   conv(x, s*W) >= -s*bias
so host-side we fold sign into the (still ternary) weights and compare each
output channel against a per-channel threshold with one DVE is_ge op.

Sharding: data-parallel over batch N=32 -> 4 images per core on 8 cores.
Weights/bias/sign are tiny and replicated.

Layout trick: each padded 66x66 image is stored row-contiguous ([128 cin
partitions, 66*66] per image), so a 512-wide output block is one contiguous
stream spanning ~7.8 rows. The conv's 9 taps become pure element offsets
(kh*66 + kw-1). Taps pair into fp8 DoubleRow matmuls (2 MACs/cell/cycle,
effective contraction 256): rhs AP [128, 2(pair stride), 512(stream)],
weights AP [128, 2(stride 256), 128]. Pairs: (kh=0,kw)+(kh=1,kw) at stride
66 for kw in 0..2, and (kh=2,kw=0)+(kh=2,kw=2) at stride 2; only (kh=2,kw=1)
remains a normal-rate matmul -> 5 matmuls per 512-wide output tile instead
of 9. Row-boundary columns (stored col 0 / 65) compute junk that the host
strips (~3% waste).

Per (img, cout_half): 8 PSUM tiles of 512 + one of 256 (valid outputs end at
stored elem 4224); weight-stationary tap-outer order in sub-groups; one DVE
tensor_scalar(is_ge) per tile drains PSUM -> SBUF as 0/1 fp32; output DMAs
batched ~1 MiB.
"""

import numpy as np
import ml_dtypes

N, CIN, H, W = 32, 128, 64, 64
COUT, KH, KW = 256, 3, 3
N_CORES = 8
IMGS = N // N_CORES          # 4 images per core
WS = W + 2                   # 66: stored row width ([0][64 data][0])
ROWS = H + 2                 # 66 stored rows (top/bottom zero rows)
XROW = ROWS * WS             # 4356 elements per image per partition
GF, GT = 16, 512             # guard zeros before/after the image block
XTOT = GF + IMGS * XROW + GT
NB = 512                     # full PSUM tile free dim (one bank)
# PSUM tile sizes per (img, half): valid outputs end at 64*66=4224
NBS = [512] * 8 + [128]      # coverage 4224 exactly
OUT_N = sum(NBS)             # 4352 stored output elems per (img, half)
NHALF = COUT // 128          # 2 cout halves
SUBGROUPS = ((0, 5), (5, 4))  # (start tile, n tiles) weight-stationary spans
NPAIR = 4                    # DoubleRow tap pairs per tile
FP8 = ml_dtypes.float8_e4m3  # TRN float8e4; {-1,0,1} and {0,1} are exact
XCHUNK = 42 * WS             # first-chunk rows of img 0 (covers tiles 0..4)

_CACHE = {}
LAST_RESULT = None           # BassKernelResults of the last run (for profiling)


def _build():
    import concourse.bass as bass
    import concourse.mybir as mybir
    from concourse import bacc
    from concourse.tile import TileContext

    dt = mybir.dt
    nc = bacc.Bacc()
    xp = nc.dram_tensor("xp", [IMGS, 128, XROW], dt.float8e4, kind="ExternalInput")
    # pair weights: [cin, pair, 2, cout] flattened; pairs 0..2 = (kh0,kh1) per
    # kw, pair 3 = ((kh2,kw0),(kh2,kw2))
    wtp = nc.dram_tensor(
        "wtp", [128, NPAIR * 2 * COUT], dt.float8e4, kind="ExternalInput"
    )
    # the lone single tap (kh2,kw1): [cin, cout]
    wts = nc.dram_tensor("wts", [128, COUT], dt.float8e4, kind="ExternalInput")
    th = nc.dram_tensor("th", [128, NHALF], dt.float32, kind="ExternalInput")
    out = nc.dram_tensor(
        "out", [IMGS, NHALF, 128, OUT_N], dt.float32, kind="ExternalOutput"
    )

    DR = mybir.MatmulPerfMode.DoubleRow
    # (pair rhs offset, pair stride) per DoubleRow pair index
    PAIR_GEOM = [(-1, WS), (0, WS), (1, WS), (2 * WS - 1, 2)]
    SINGLE_OFF = 2 * WS  # (kh2, kw1)

    with TileContext(nc) as tc:
        with (
            tc.tile_pool(name="const", bufs=1) as cpool,
            tc.tile_pool(name="xin", bufs=1) as xpool,
            tc.tile_pool(name="psum", bufs=8, space="PSUM") as ppool,
            tc.tile_pool(name="outb", bufs=3) as opool,
        ):
            wtp_t = cpool.tile([128, NPAIR * 2 * COUT], dt.float8e4, tag="wtp")
            nc.sync.dma_start(out=wtp_t[:], in_=wtp[:])
            wts_t = cpool.tile([128, COUT], dt.float8e4, tag="wts")
            nc.scalar.dma_start(out=wts_t[:], in_=wts[:])
            th_t = cpool.tile([128, NHALF], dt.float32, tag="th")
            nc.scalar.dma_start(out=th_t[:], in_=th[:])

            xs_t = xpool.tile([128, XTOT], dt.float8e4, tag="xs")
            xs = xs_t[:]
            # zero the guard regions (junk reads must not hit fp8 NaN bytes)
            nc.gpsimd.memset(xs[:, :GF], 0)
            nc.gpsimd.memset(xs[:, GF + IMGS * XROW :], 0)
            # img 0 split in two chunks so compute starts sooner; imgs 1..3
            # issued on the scalar HWDGE queue in parallel
            nc.sync.dma_start(out=xs[:, GF : GF + XCHUNK], in_=xp[0][:, :XCHUNK])
            nc.sync.dma_start(
                out=xs[:, GF + XCHUNK : GF + XROW], in_=xp[0][:, XCHUNK:]
            )
            for i in range(1, IMGS):
                nc.scalar.dma_start(
                    out=xs[:, GF + i * XROW : GF + (i + 1) * XROW], in_=xp[i]
                )

            # Warm the PE clock (HAM un-throttle needs ~3.4us of sustained
            # activity) with zero-weight matmuls on the zeroed guard region
            # while the input DMAs are still in flight.
            wz_t = cpool.tile([128, 128], dt.float8e4, tag="wz")
            nc.gpsimd.memset(wz_t[:], 0)
            pd = ppool.tile([128, NB], dt.float32, tag="ps", name="pd")
            for _ in range(9):
                nc.tensor.matmul(
                    pd[:], wz_t[:], xs[:, XTOT - NB :], start=True, stop=True
                )

            xten, xap0 = xs.tensor, list(xs.ap[0])
            wpten, wpap0 = wtp_t[:].tensor, list(wtp_t[:].ap[0])

            def rhs_pair(base, p, nb):
                off, stride = PAIR_GEOM[p]
                return bass.AP(xten, base + off, [xap0, [stride, 2], [1, nb]])

            def lhs_pair(p, h):
                return bass.AP(
                    wpten, p * 2 * COUT + h * 128, [wpap0, [COUT, 2], [1, 128]]
                )

            starts = [sum(NBS[:t]) for t in range(len(NBS))]
            for img in range(IMGS):
                xbase = GF + img * XROW
                for h in range(NHALF):
                    last_blk = img == IMGS - 1 and h == NHALF - 1
                    for sg_start, sg_n in SUBGROUPS:
                        tls = list(range(sg_start, sg_start + sg_n))
                        ow = sum(NBS[t] for t in tls)
                        ot = opool.tile([128, ow], dt.float32, tag="ot", name="ot")
                        ps = [
                            ppool.tile([128, NBS[t]], dt.float32, tag="ps", name="ps")
                            for t in tls
                        ]
                        for j, t in enumerate(tls):
                            # tiny zero-weight start matmul pays the whole-bank
                            # has_written clear without stalling a real matmul;
                            # elem 0 accumulates 0, elems 1.. overwrite-on-clear
                            nc.tensor.matmul(
                                ps[j][:, 0:1],
                                wz_t[:],
                                xs[:, XTOT - 1 :],
                                start=True,
                                stop=False,
                                skip_group_check=True,
                            )
                        for p in range(NPAIR):
                            wap = lhs_pair(p, h)
                            for j, t in enumerate(tls):
                                nc.tensor.matmul(
                                    ps[j][:],
                                    wap,
                                    rhs_pair(xbase + starts[t], p, NBS[t]),
                                    perf_mode=DR,
                                    start=False,
                                    stop=False,
                                    skip_group_check=True,
                                )
                        wap = wts_t[:, h * 128 : (h + 1) * 128]
                        for j, t in enumerate(tls):
                            b = xbase + starts[t] + SINGLE_OFF
                            nc.tensor.matmul(
                                ps[j][:],
                                wap,
                                xs[:, b : b + NBS[t]],
                                start=False,
                                stop=True,
                                skip_group_check=True,
                            )
                        ob = 0
                        for j, t in enumerate(tls):
                            nc.vector.tensor_scalar(
                                out=ot[:, ob : ob + NBS[t]],
                                in0=ps[j][:],
                                scalar1=th_t[:, h : h + 1],
                                scalar2=None,
                                op0=mybir.AluOpType.is_ge,
                            )
                            ob += NBS[t]
                        dst = out[img, h][:, starts[sg_start] : starts[sg_start] + ow]
                        if last_blk and sg_n > 1:
                            # fine-grained final DMAs so the kernel tail is short
                            ob = 0
                            for j, t in enumerate(tls):
                                eng = nc.sync if j % 2 == 0 else nc.scalar
                                eng.dma_start(
                                    out=dst[:, ob : ob + NBS[t]],
                                    in_=ot[:, ob : ob + NBS[t]],
                                )
                                ob += NBS[t]
                        else:
                            nc.sync.dma_start(out=dst, in_=ot[:])
    nc.finalize()
    return nc


def kernel(x, weight, bias, sign):
    global LAST_RESULT
    from concourse.bass_utils import run_bass_kernel_spmd

    if "nc" not in _CACHE:
        _CACHE["nc"] = _build()
    nc = _CACHE["nc"]

    sign_v = np.asarray(sign, dtype=np.float32).reshape(COUT)
    wsig = np.asarray(weight, dtype=np.float32) * sign_v[:, None, None, None]
    # wsig[cout, cin, kh, kw] -> pairs [cin, pair, 2, cout]
    wtp_host = np.zeros((CIN, NPAIR, 2, COUT), dtype=np.float32)
    for kw in range(KW):  # pairs 0..2: (kh0, kw), (kh1, kw)
        wtp_host[:, kw, 0] = wsig[:, :, 0, kw].T
        wtp_host[:, kw, 1] = wsig[:, :, 1, kw].T
    wtp_host[:, 3, 0] = wsig[:, :, 2, 0].T  # pair 3: (kh2,kw0),(kh2,kw2)
    wtp_host[:, 3, 1] = wsig[:, :, 2, 2].T
    wtp_host = wtp_host.reshape(CIN, NPAIR * 2 * COUT).astype(FP8)
    wts_host = np.ascontiguousarray(wsig[:, :, 2, 1].T).astype(FP8)
    th_host = np.ascontiguousarray(
        (-sign_v * np.asarray(bias, dtype=np.float32)).reshape(NHALF, 128).T
    ).astype(np.float32)

    x = np.asarray(x, dtype=np.float32)
    in_maps = []
    for c in range(N_CORES):
        xpad = np.zeros((IMGS, CIN, ROWS, WS), dtype=FP8)
        xpad[:, :, 1 : H + 1, 1 : W + 1] = x[c * IMGS : (c + 1) * IMGS]
        in_maps.append(
            {
                "xp": xpad.reshape(IMGS, CIN, XROW),
                "wtp": wtp_host,
                "wts": wts_host,
                "th": th_host,
            }
        )

    res = run_bass_kernel_spmd(nc, in_maps, core_ids=list(range(N_CORES)))
    LAST_RESULT = res
    # strip stored-row junk: [img, h, co, OUT_N] -> rows 0..63, cols 1..64
    full = np.concatenate(
        [
            r["out"][..., : H * WS]
            .reshape(IMGS, COUT, H, WS)[..., 1 : W + 1]
            for r in res.results
        ],
        axis=0,
    )
    return np.ascontiguousarray(full)


# revision 27
# speedup vs baseline: 1.0351x; 1.0351x over previous
"""BinConv2dEval Trainium2 kernel (fp8 DoubleRow version).

y = conv2d(x, W, stride 1, pad 1) + bias ; out = (round(y) * sign >= 0) ? 1 : 0

All values are integers (x in {0,1}, W in {-1,0,1}, bias integer), so round()
is a no-op and everything is exact in fp8e4 matmuls with fp32 PSUM
accumulation. Folding: with s = sign[c] in {+-1},
    (conv + bias) * s >= 0   <=>   conv(x, s*W) >= -s*bias
so host-side we fold sign into the (still ternary) weights and compare each
output channel against a per-channel threshold with one DVE is_ge op.

Sharding: data-parallel over batch N=32 -> 4 images per core on 8 cores.
Weights/bias/sign are tiny and replicated.

Layout trick: each padded 66x66 image is stored row-contiguous ([128 cin
partitions, 66*66] per image), so a 512-wide output block is one contiguous
stream spanning ~7.8 rows. The conv's 9 taps become pure element offsets
(kh*66 + kw-1). Taps pair into fp8 DoubleRow matmuls (2 MACs/cell/cycle,
effective contraction 256): rhs AP [128, 2(pair stride), 512(stream)],
weights AP [128, 2(stride 256), 128]. Pairs: (kh=0,kw)+(kh=1,kw) at stride
66 for kw in 0..2, and (kh=2,kw=0)+(kh=2,kw=2) at stride 2; only (kh=2,kw=1)
remains a normal-rate matmul -> 5 matmuls per 512-wide output tile instead
of 9. Row-boundary columns (stored col 0 / 65) compute junk that the host
strips (~3% waste).

Per (img, cout_half): 8 PSUM tiles of 512 + one of 256 (valid outputs end at
stored elem 4224); weight-stationary tap-outer order in sub-groups; one DVE
tensor_scalar(is_ge) per tile drains PSUM -> SBUF as 0/1 fp32; output DMAs
batched ~1 MiB.
"""

import numpy as np
import ml_dtypes

N, CIN, H, W = 32, 128, 64, 64
COUT, KH, KW = 256, 3, 3
N_CORES = 8
IMGS = N // N_CORES          # 4 images per core
WS = W + 2                   # 66: stored row width ([0][64 data][0])
ROWS = H + 2                 # 66 stored rows (top/bottom zero rows)
XROW = ROWS * WS             # 4356 elements per image per partition
GF, GT = 16, 512             # guard zeros before/after the image block
XTOT = GF + IMGS * XROW + GT
NB = 512                     # full PSUM tile free dim (one bank)
# PSUM tile sizes per (img, half): valid outputs end at 64*66=4224
NBS = [512] * 8 + [128]      # coverage 4224 exactly
OUT_N = sum(NBS)             # 4352 stored output elems per (img, half)
NHALF = COUT // 128          # 2 cout halves
SUBGROUPS = ((0, 5), (5, 4))  # (start tile, n tiles) weight-stationary spans
NPAIR = 4                    # DoubleRow tap pairs per tile
FP8 = ml_dtypes.float8_e4m3  # TRN float8e4; {-1,0,1} and {0,1} are exact
XCHUNK = 42 * WS             # first-chunk rows of img 0 (covers tiles 0..4)

_CACHE = {}
LAST_RESULT = None           # BassKernelResults of the last run (for profiling)


def _build():
    import concourse.bass as bass
    import concourse.mybir as mybir
    from concourse import bacc
    from concourse.tile import TileContext

    dt = mybir.dt
    nc = bacc.Bacc()
    xp = nc.dram_tensor("xp", [IMGS, 128, XROW], dt.float8e4, kind="ExternalInput")
    # pair weights: [cin, pair, 2, cout] flattened; pairs 0..2 = (kh0,kh1) per
    # kw, pair 3 = ((kh2,kw0),(kh2,kw2))
    wtp = nc.dram_tensor(
        "wtp", [128, NPAIR * 2 * COUT], dt.float8e4, kind="ExternalInput"
    )
    # the lone single tap (kh2,kw1): [cin, cout]
    wts = nc.dram_tensor("wts", [128, COUT], dt.float8e4, kind="ExternalInput")
    th = nc.dram_tensor("th", [128, NHALF], dt.float32, kind="ExternalInput")
    out = nc.dram_tensor(
        "out", [IMGS, NHALF, 128, OUT_N], dt.float32, kind="ExternalOutput"
    )

    DR = mybir.MatmulPerfMode.DoubleRow
    # (pair rhs offset, pair stride) per DoubleRow pair index
    PAIR_GEOM = [(-1, WS), (0, WS), (1, WS), (2 * WS - 1, 2)]
    SINGLE_OFF = 2 * WS  # (kh2, kw1)

    with TileContext(nc) as tc:
        with (
            tc.tile_pool(name="const", bufs=1) as cpool,
            tc.tile_pool(name="xin", bufs=1) as xpool,
            tc.tile_pool(name="psum", bufs=8, space="PSUM") as ppool,
            tc.tile_pool(name="outb", bufs=3) as opool,
        ):
            wtp_t = cpool.tile([128, NPAIR * 2 * COUT], dt.float8e4, tag="wtp")
            nc.sync.dma_start(out=wtp_t[:], in_=wtp[:])
            wts_t = cpool.tile([128, COUT], dt.float8e4, tag="wts")
            nc.scalar.dma_start(out=wts_t[:], in_=wts[:])
            th_t = cpool.tile([128, NHALF], dt.float32, tag="th")
            nc.scalar.dma_start(out=th_t[:], in_=th[:])

            xs_t = xpool.tile([128, XTOT], dt.float8e4, tag="xs")
            xs = xs_t[:]
            # zero the guard regions (junk reads must not hit fp8 NaN bytes)
            nc.gpsimd.memset(xs[:, :GF], 0)
            nc.gpsimd.memset(xs[:, GF + IMGS * XROW :], 0)
            # img 0 split in two chunks so compute starts sooner; imgs 1..3
            # issued on the scalar HWDGE queue in parallel
            nc.sync.dma_start(out=xs[:, GF : GF + XCHUNK], in_=xp[0][:, :XCHUNK])
            nc.sync.dma_start(
                out=xs[:, GF + XCHUNK : GF + XROW], in_=xp[0][:, XCHUNK:]
            )
            for i in range(1, IMGS):
                nc.scalar.dma_start(
                    out=xs[:, GF + i * XROW : GF + (i + 1) * XROW], in_=xp[i]
                )

            # Warm the PE clock (HAM un-throttle needs ~3.4us of sustained
            # activity) with zero-weight matmuls on the zeroed guard region
            # while the input DMAs are still in flight.
            wz_t = cpool.tile([128, 128], dt.float8e4, tag="wz")
            nc.gpsimd.memset(wz_t[:], 0)
            pd = ppool.tile([128, NB], dt.float32, tag="ps", name="pd")
            for _ in range(9):
                nc.tensor.matmul(
                    pd[:], wz_t[:], xs[:, XTOT - NB :], start=True, stop=True
                )

            xten, xap0 = xs.tensor, list(xs.ap[0])
            wpten, wpap0 = wtp_t[:].tensor, list(wtp_t[:].ap[0])

            def rhs_pair(base, p, nb):
                off, stride = PAIR_GEOM[p]
                return bass.AP(xten, base + off, [xap0, [stride, 2], [1, nb]])

            def lhs_pair(p, h):
                return bass.AP(
                    wpten, p * 2 * COUT + h * 128, [wpap0, [COUT, 2], [1, 128]]
                )

            starts = [sum(NBS[:t]) for t in range(len(NBS))]
            for img in range(IMGS):
                xbase = GF + img * XROW
                for h in range(NHALF):
                    last_blk = img == IMGS - 1 and h == NHALF - 1
                    for sg_start, sg_n in SUBGROUPS:
                        tls = list(range(sg_start, sg_start + sg_n))
                        ow = sum(NBS[t] for t in tls)
                        ot = opool.tile([128, ow], dt.float32, tag="ot", name="ot")
                        ps = [
                            ppool.tile([128, NBS[t]], dt.float32, tag="ps", name="ps")
                            for t in tls
                        ]
                        for p in range(NPAIR):
                            wap = lhs_pair(p, h)
                            for j, t in enumerate(tls):
                                nc.tensor.matmul(
                                    ps[j][:],
                                    wap,
                                    rhs_pair(xbase + starts[t], p, NBS[t]),
                                    perf_mode=DR,
                                    start=(p == 0),
                                    stop=False,
                                )
                        wap = wts_t[:, h * 128 : (h + 1) * 128]
                        for j, t in enumerate(tls):
                            b = xbase + starts[t] + SINGLE_OFF
                            nc.tensor.matmul(
                                ps[j][:],
                                wap,
                                xs[:, b : b + NBS[t]],
                                start=False,
                                stop=True,
                            )
                        ob = 0
                        for j, t in enumerate(tls):
                            nc.vector.tensor_scalar(
                                out=ot[:, ob : ob + NBS[t]],
                                in0=ps[j][:],
                                scalar1=th_t[:, h : h + 1],
                                scalar2=None,
                                op0=mybir.AluOpType.is_ge,
                            )
                            ob += NBS[t]
                        dst = out[img, h][:, starts[sg_start] : starts[sg_start] + ow]
                        if last_blk and sg_n > 1:
                            # fine-grained final DMAs so the kernel tail is short
                            ob = 0
                            for j, t in enumerate(tls):
                                eng = nc.sync if j % 2 == 0 else nc.scalar
                                eng.dma_start(
                                    out=dst[:, ob : ob + NBS[t]],
                                    in_=ot[:, ob : ob + NBS[t]],
                                )
                                ob += NBS[t]
                        else:
                            nc.sync.dma_start(out=dst, in_=ot[:])
    nc.finalize()
    return nc


def kernel(x, weight, bias, sign):
    global LAST_RESULT
    from concourse.bass_utils import run_bass_kernel_spmd

    if "nc" not in _CACHE:
        _CACHE["nc"] = _build()
    nc = _CACHE["nc"]

    sign_v = np.asarray(sign, dtype=np.float32).reshape(COUT)
    wsig = np.asarray(weight, dtype=np.float32) * sign_v[:, None, None, None]
    # wsig[cout, cin, kh, kw] -> pairs [cin, pair, 2, cout]
    wtp_host = np.zeros((CIN, NPAIR, 2, COUT), dtype=np.float32)
    for kw in range(KW):  # pairs 0..2: (kh0, kw), (kh1, kw)
        wtp_host[:, kw, 0] = wsig[:, :, 0, kw].T
        wtp_host[:, kw, 1] = wsig[:, :, 1, kw].T
    wtp_host[:, 3, 0] = wsig[:, :, 2, 0].T  # pair 3: (kh2,kw0),(kh2,kw2)
    wtp_host[:, 3, 1] = wsig[:, :, 2, 2].T
    wtp_host = wtp_host.reshape(CIN, NPAIR * 2 * COUT).astype(FP8)
    wts_host = np.ascontiguousarray(wsig[:, :, 2, 1].T).astype(FP8)
    th_host = np.ascontiguousarray(
        (-sign_v * np.asarray(bias, dtype=np.float32)).reshape(NHALF, 128).T
    ).astype(np.float32)

    x = np.asarray(x, dtype=np.float32)
    in_maps = []
    for c in range(N_CORES):
        xpad = np.zeros((IMGS, CIN, ROWS, WS), dtype=FP8)
        xpad[:, :, 1 : H + 1, 1 : W + 1] = x[c * IMGS : (c + 1) * IMGS]
        in_maps.append(
            {
                "xp": xpad.reshape(IMGS, CIN, XROW),
                "wtp": wtp_host,
                "wts": wts_host,
                "th": th_host,
            }
        )

    res = run_bass_kernel_spmd(nc, in_maps, core_ids=list(range(N_CORES)))
    LAST_RESULT = res
    # strip stored-row junk: [img, h, co, OUT_N] -> rows 0..63, cols 1..64
    full = np.concatenate(
        [
            r["out"][..., : H * WS]
            .reshape(IMGS, COUT, H, WS)[..., 1 : W + 1]
            for r in res.results
        ],
        axis=0,
    )
    return np.ascontiguousarray(full)


# revision 28
# speedup vs baseline: 1.0465x; 1.0110x over previous
"""BinConv2dEval Trainium2 kernel (fp8 DoubleRow version).

y = conv2d(x, W, stride 1, pad 1) + bias ; out = (round(y) * sign >= 0) ? 1 : 0

All values are integers (x in {0,1}, W in {-1,0,1}, bias integer), so round()
is a no-op and everything is exact in fp8e4 matmuls with fp32 PSUM
accumulation. Folding: with s = sign[c] in {+-1},
    (conv + bias) * s >= 0   <=>   conv(x, s*W) >= -s*bias
so host-side we fold sign into the (still ternary) weights and compare each
output channel against a per-channel threshold with one DVE is_ge op.

Sharding: data-parallel over batch N=32 -> 4 images per core on 8 cores.
Weights/bias/sign are tiny and replicated.

Layout trick: each padded 66x66 image is stored row-contiguous ([128 cin
partitions, 66*66] per image), so a 512-wide output block is one contiguous
stream spanning ~7.8 rows. The conv's 9 taps become pure element offsets
(kh*66 + kw-1). Taps pair into fp8 DoubleRow matmuls (2 MACs/cell/cycle,
effective contraction 256): rhs AP [128, 2(pair stride), 512(stream)],
weights AP [128, 2(stride 256), 128]. Pairs: (kh=0,kw)+(kh=1,kw) at stride
66 for kw in 0..2, and (kh=2,kw=0)+(kh=2,kw=2) at stride 2; only (kh=2,kw=1)
remains a normal-rate matmul -> 5 matmuls per 512-wide output tile instead
of 9. Row-boundary columns (stored col 0 / 65) compute junk that the host
strips (~3% waste).

Per (img, cout_half): 8 PSUM tiles of 512 + one of 256 (valid outputs end at
stored elem 4224); weight-stationary tap-outer order in sub-groups; one DVE
tensor_scalar(is_ge) per tile drains PSUM -> SBUF as 0/1 fp32; output DMAs
batched ~1 MiB.
"""

import numpy as np
import ml_dtypes

N, CIN, H, W = 32, 128, 64, 64
COUT, KH, KW = 256, 3, 3
N_CORES = 8
IMGS = N // N_CORES          # 4 images per core
WS = W + 2                   # 66: stored row width ([0][64 data][0])
ROWS = H + 2                 # 66 stored rows (top/bottom zero rows)
XROW = ROWS * WS             # 4356 elements per image per partition
GF, GT = 16, 512             # guard zeros before/after the image block
XTOT = GF + IMGS * XROW + GT
NB = 512                     # full PSUM tile free dim (one bank)
# PSUM tile sizes per (img, half): valid outputs end at 64*66=4224
NBS = [512] * 8 + [128]      # coverage 4224 exactly
OUT_N = sum(NBS)             # 4352 stored output elems per (img, half)
NHALF = COUT // 128          # 2 cout halves
SUBGROUPS = ((0, 5), (5, 4))  # (start tile, n tiles) weight-stationary spans
NPAIR = 4                    # DoubleRow tap pairs per tile
FP8 = ml_dtypes.float8_e4m3  # TRN float8e4; {-1,0,1} and {0,1} are exact
XCHUNK = 42 * WS             # first-chunk rows of img 0 (covers tiles 0..4)

_CACHE = {}
LAST_RESULT = None           # BassKernelResults of the last run (for profiling)


def _build():
    import concourse.bass as bass
    import concourse.mybir as mybir
    from concourse import bacc
    from concourse.tile import TileContext

    dt = mybir.dt
    nc = bacc.Bacc()
    xp = nc.dram_tensor("xp", [IMGS, 128, XROW], dt.float8e4, kind="ExternalInput")
    # pair weights: [cin, pair, 2, cout] flattened; pairs 0..2 = (kh0,kh1) per
    # kw, pair 3 = ((kh2,kw0),(kh2,kw2))
    wtp = nc.dram_tensor(
        "wtp", [128, NPAIR * 2 * COUT], dt.float8e4, kind="ExternalInput"
    )
    # the lone single tap (kh2,kw1): [cin, cout]
    wts = nc.dram_tensor("wts", [128, COUT], dt.float8e4, kind="ExternalInput")
    th = nc.dram_tensor("th", [128, NHALF], dt.float32, kind="ExternalInput")
    out = nc.dram_tensor(
        "out", [IMGS, NHALF, 128, OUT_N], dt.float32, kind="ExternalOutput"
    )

    DR = mybir.MatmulPerfMode.DoubleRow
    # (pair rhs offset, pair stride) per DoubleRow pair index
    PAIR_GEOM = [(-1, WS), (0, WS), (1, WS), (2 * WS - 1, 2)]
    SINGLE_OFF = 2 * WS  # (kh2, kw1)

    with TileContext(nc) as tc:
        with (
            tc.tile_pool(name="const", bufs=1) as cpool,
            tc.tile_pool(name="xin", bufs=1) as xpool,
            tc.tile_pool(name="psum", bufs=8, space="PSUM") as ppool,
            tc.tile_pool(name="outb", bufs=3) as opool,
        ):
            wtp_t = cpool.tile([128, NPAIR * 2 * COUT], dt.float8e4, tag="wtp")
            nc.sync.dma_start(out=wtp_t[:], in_=wtp[:])
            wts_t = cpool.tile([128, COUT], dt.float8e4, tag="wts")
            nc.scalar.dma_start(out=wts_t[:], in_=wts[:])
            th_t = cpool.tile([128, NHALF], dt.float32, tag="th")
            nc.scalar.dma_start(out=th_t[:], in_=th[:])

            xs_t = xpool.tile([128, XTOT], dt.float8e4, tag="xs")
            xs = xs_t[:]
            # zero the guard regions (junk reads must not hit fp8 NaN bytes)
            nc.gpsimd.memset(xs[:, :GF], 0)
            nc.gpsimd.memset(xs[:, GF + IMGS * XROW :], 0)
            # img 0 split in two chunks so compute starts sooner; imgs 1..3
            # issued on the scalar HWDGE queue in parallel
            nc.sync.dma_start(out=xs[:, GF : GF + XCHUNK], in_=xp[0][:, :XCHUNK])
            nc.sync.dma_start(
                out=xs[:, GF + XCHUNK : GF + XROW], in_=xp[0][:, XCHUNK:]
            )
            for i in range(1, IMGS):
                nc.scalar.dma_start(
                    out=xs[:, GF + i * XROW : GF + (i + 1) * XROW], in_=xp[i]
                )

            # Warm the PE clock (HAM un-throttle needs ~3.4us of sustained
            # activity) with zero-weight matmuls on the zeroed guard region
            # while the input DMAs are still in flight.
            wz_t = cpool.tile([128, 128], dt.float8e4, tag="wz")
            nc.gpsimd.memset(wz_t[:], 0)
            pd = ppool.tile([128, NB], dt.float32, tag="ps", name="pd")
            for _ in range(9):
                nc.tensor.matmul(
                    pd[:], wz_t[:], xs[:, XTOT - NB :], start=True, stop=True
                )

            xten, xap0 = xs.tensor, list(xs.ap[0])
            wpten, wpap0 = wtp_t[:].tensor, list(wtp_t[:].ap[0])

            def rhs_pair(base, p, nb):
                off, stride = PAIR_GEOM[p]
                return bass.AP(xten, base + off, [xap0, [stride, 2], [1, nb]])

            def lhs_pair(p, h):
                return bass.AP(
                    wpten, p * 2 * COUT + h * 128, [wpap0, [COUT, 2], [1, 128]]
                )

            starts = [sum(NBS[:t]) for t in range(len(NBS))]
            for img in range(IMGS):
                xbase = GF + img * XROW
                for h in range(NHALF):
                    last_blk = img == IMGS - 1 and h == NHALF - 1
                    for sg_start, sg_n in SUBGROUPS:
                        tls = list(range(sg_start, sg_start + sg_n))
                        ow = sum(NBS[t] for t in tls)
                        ot = opool.tile([128, ow], dt.float32, tag="ot", name="ot")
                        ps = [
                            ppool.tile([128, NBS[t]], dt.float32, tag="ps", name="ps")
                            for t in tls
                        ]
                        wap = wts_t[:, h * 128 : (h + 1) * 128]
                        for j, t in enumerate(tls):
                            b = xbase + starts[t] + SINGLE_OFF
                            nc.tensor.matmul(
                                ps[j][:],
                                wap,
                                xs[:, b : b + NBS[t]],
                                start=True,
                                stop=False,
                            )
                        for p in range(NPAIR):
                            wap = lhs_pair(p, h)
                            for j, t in enumerate(tls):
                                nc.tensor.matmul(
                                    ps[j][:],
                                    wap,
                                    rhs_pair(xbase + starts[t], p, NBS[t]),
                                    perf_mode=DR,
                                    start=False,
                                    stop=(p == NPAIR - 1),
                                )
                        ob = 0
                        for j, t in enumerate(tls):
                            nc.vector.tensor_scalar(
                                out=ot[:, ob : ob + NBS[t]],
                                in0=ps[j][:],
                                scalar1=th_t[:, h : h + 1],
                                scalar2=None,
                                op0=mybir.AluOpType.is_ge,
                            )
                            ob += NBS[t]
                        dst = out[img, h][:, starts[sg_start] : starts[sg_start] + ow]
                        if last_blk and sg_n > 1:
                            # fine-grained final DMAs so the kernel tail is short
                            ob = 0
                            for j, t in enumerate(tls):
                                eng = nc.sync if j % 2 == 0 else nc.scalar
                                eng.dma_start(
                                    out=dst[:, ob : ob + NBS[t]],
                                    in_=ot[:, ob : ob + NBS[t]],
                                )
                                ob += NBS[t]
                        else:
                            nc.sync.dma_start(out=dst, in_=ot[:])
    nc.finalize()
    return nc


def kernel(x, weight, bias, sign):
    global LAST_RESULT
    from concourse.bass_utils import run_bass_kernel_spmd

    if "nc" not in _CACHE:
        _CACHE["nc"] = _build()
    nc = _CACHE["nc"]

    sign_v = np.asarray(sign, dtype=np.float32).reshape(COUT)
    wsig = np.asarray(weight, dtype=np.float32) * sign_v[:, None, None, None]
    # wsig[cout, cin, kh, kw] -> pairs [cin, pair, 2, cout]
    wtp_host = np.zeros((CIN, NPAIR, 2, COUT), dtype=np.float32)
    for kw in range(KW):  # pairs 0..2: (kh0, kw), (kh1, kw)
        wtp_host[:, kw, 0] = wsig[:, :, 0, kw].T
        wtp_host[:, kw, 1] = wsig[:, :, 1, kw].T
    wtp_host[:, 3, 0] = wsig[:, :, 2, 0].T  # pair 3: (kh2,kw0),(kh2,kw2)
    wtp_host[:, 3, 1] = wsig[:, :, 2, 2].T
    wtp_host = wtp_host.reshape(CIN, NPAIR * 2 * COUT).astype(FP8)
    wts_host = np.ascontiguousarray(wsig[:, :, 2, 1].T).astype(FP8)
    th_host = np.ascontiguousarray(
        (-sign_v * np.asarray(bias, dtype=np.float32)).reshape(NHALF, 128).T
    ).astype(np.float32)

    x = np.asarray(x, dtype=np.float32)
    in_maps = []
    for c in range(N_CORES):
        xpad = np.zeros((IMGS, CIN, ROWS, WS), dtype=FP8)
        xpad[:, :, 1 : H + 1, 1 : W + 1] = x[c * IMGS : (c + 1) * IMGS]
        in_maps.append(
            {
                "xp": xpad.reshape(IMGS, CIN, XROW),
                "wtp": wtp_host,
                "wts": wts_host,
                "th": th_host,
            }
        )

    res = run_bass_kernel_spmd(nc, in_maps, core_ids=list(range(N_CORES)))
    LAST_RESULT = res
    # strip stored-row junk: [img, h, co, OUT_N] -> rows 0..63, cols 1..64
    full = np.concatenate(
        [
            r["out"][..., : H * WS]
            .reshape(IMGS, COUT, H, WS)[..., 1 : W + 1]
            for r in res.results
        ],
        axis=0,
    )
    return np.ascontiguousarray(full)
